# revision 22
# baseline (speedup 1.0000x reference)
"""Trainium2 Bass kernel for DecoderRNNWithAttention.

Data-parallel over batch (8 samples/core x 8 cores), weights replicated,
bf16 matmul inputs with f32 PSUM accumulation, elementwise in f32.
Per core: startup (enc load/cast, xbar transpose, att1 GEMM, mean/h0/c0,
embedding gather + emb-gate GEMM staged to DRAM), 19 recurrence steps,
then a batched fc projection over all (b,t) rows with streamed fc weights.
"""
import numpy as np
import ml_dtypes
from contextlib import ExitStack

import concourse.bass as bass
import concourse.bacc as bacc
import concourse.tile as tile
import concourse.mybir as mybir
from concourse.masks import make_identity

BF = mybir.dt.bfloat16
F32 = mybir.dt.float32
I32 = mybir.dt.int32
I8 = mybir.dt.int8
AF = mybir.ActivationFunctionType
ALU = mybir.AluOpType

B, P, ENC, H, A, E, V, T = 64, 196, 2048, 512, 512, 512, 10000, 20
NC_N = 8
S = B // NC_N          # 8 samples per core
T1 = T - 1             # 19 steps
SR = S * T1            # 152 (b, t) rows, b-major t-fast
P1 = P - 128           # 68


def _bcast(d_ap, nparts):
    return bass.AP(tensor=d_ap.tensor, offset=d_ap.offset,
                   ap=[[0, nparts]] + [list(p) for p in d_ap.ap])


def build(full_att_b: float, nz_fcb: bool):
    nc = bacc.Bacc("TRN2", target_bir_lowering=False)

    enc_d = nc.dram_tensor("enc", [S, P, ENC], F32, kind="ExternalInput")
    caps_d = nc.dram_tensor("caps", [SR, 1], I32, kind="ExternalInput")
    masks_d = nc.dram_tensor("masks", [S, T1], F32, kind="ExternalInput")
    masktr_d = nc.dram_tensor("masktr", [T1, S], BF, kind="ExternalInput")
    masksi_d = nc.dram_tensor("masksi", [S, T1], I32, kind="ExternalInput")
    masktri_d = nc.dram_tensor("masktri", [T1, S], I8, kind="ExternalInput")
    maskrow_d = nc.dram_tensor("maskrow", [1, SR], BF, kind="ExternalInput")
    embw_d = nc.dram_tensor("embw", [V, E], F32, kind="ExternalInput")
    encattwT_d = nc.dram_tensor("encattwT", [ENC, A], BF, kind="ExternalInput")
    encattb_d = nc.dram_tensor("encattb", [128, A // 128], F32, kind="ExternalInput")
    hidattT_d = nc.dram_tensor("hidattT", [H, A], BF, kind="ExternalInput")
    hidattb_d = nc.dram_tensor("hidattb", [128, A // 128], F32, kind="ExternalInput")
    wfull_d = nc.dram_tensor("wfull", [128, A // 128], BF, kind="ExternalInput")
    inithcT_d = nc.dram_tensor("inithcT", [ENC, 2 * H], BF, kind="ExternalInput")
    initb_d = nc.dram_tensor("initb", [2 * H], F32, kind="ExternalInput")
    wembT_d = nc.dram_tensor("wembT", [E, 4 * H], BF, kind="ExternalInput")
    bcomb_d = nc.dram_tensor("bcomb", [4 * H], F32, kind="ExternalInput")
    wencT_d = nc.dram_tensor("wencT", [ENC, 4 * H], BF, kind="ExternalInput")
    whcomb_d = nc.dram_tensor("whcomb", [H, 2 * ENC], BF, kind="ExternalInput")
    fcT_d = nc.dram_tensor("fcT", [H, V], BF, kind="ExternalInput")
    fcb_d = nc.dram_tensor("fcb", [1, V], BF, kind="ExternalInput")

    preds_d = nc.dram_tensor("preds", [SR, V], F32, kind="ExternalOutput")
    alph_d = nc.dram_tensor("alph", [S, T1, P], F32, kind="ExternalOutput")

    rr = lambda d: d[:].rearrange("(kt q) n -> q kt n", q=128)

    with tile.TileContext(nc) as tc, ExitStack() as top:
        glob = top.enter_context(tc.tile_pool(name="glob", bufs=1))
        dramp = top.enter_context(tc.tile_pool(name="dramp", bufs=1, space="DRAM"))
        ident = glob.tile([128, 128], BF, name="ident")
        make_identity(nc, ident)
        H_allT = glob.tile([128, 4, SR], BF, name="H_allT")
        ge_dram = dramp.tile([SR, 4 * H], BF, name="ge_dram")
        ge_view = ge_dram[:].rearrange("(b t) n -> b t n", t=T1)

        with ExitStack() as rctx:  # pools that live startup..recurrence-end
            pe = rctx.enter_context(tc.tile_pool(name="pe", bufs=1))
            enc_sb = pe.tile([128, 2 * S, ENC], BF, name="enc_sb")
            att1T = pe.tile([128, 4, S * P], BF, name="att1T")
            hT = pe.tile([128, 4, S], BF, name="hT")
            c_st = pe.tile([S, H], F32, name="c_st")
            expcol = pe.tile([128, 2, S], BF, name="expcol")
            x_encT = pe.tile([128, 16, S], BF, name="x_encT")
            maskTb = pe.tile([128, T1, S], I8, name="maskTb")
            maskA = pe.tile([4, T1], F32, name="maskA")
            maskB = pe.tile([4, T1], F32, name="maskB")
            wfu = pe.tile([128, 4, 1], BF, name="wfu")
            hab = pe.tile([128, A // 128], F32, name="hab")
            # block-diag alpha lhsT per 4-sample group: [q, kt(8), g(2), b'(4)]
            bd4 = pe.tile([128, 8, 2, 4], BF, name="bd4")
            nc.vector.memset(bd4[:], 0.0)

            nc.sync.dma_start(maskTb[:], _bcast(masktri_d[:], 128))
            nc.sync.dma_start(maskA[:], masks_d[0:4, :])
            nc.sync.dma_start(maskB[:], masks_d[4:8, :])
            nc.sync.dma_start(wfu[:], wfull_d[:, :, None])
            nc.sync.dma_start(hab[:], hidattb_d[:])
            nc.vector.memset(enc_sb[:], 0.0)
            nc.vector.memset(expcol[:], 0.0)

            # ---------- Phase A1: enc load/cast, encT halves, att1 ----------
            with ExitStack() as ph:
                stg = ph.enter_context(tc.tile_pool(name="stg", bufs=2))
                stA = ph.enter_context(tc.tile_pool(name="stA", bufs=1))
                psA = ph.enter_context(tc.tile_pool(name="psA", bufs=4, space="PSUM"))

                for b in range(S):
                    for kp in range(2):
                        cnt = 128 if kp == 0 else P1
                        for cq in range(4):
                            st = stg.tile([128, 512], F32, tag="encstage")
                            nc.sync.dma_start(
                                st[0:cnt, :],
                                enc_d[b, kp * 128:kp * 128 + cnt,
                                      cq * 512:(cq + 1) * 512])
                            nc.vector.tensor_copy(
                                enc_sb[0:cnt, 2 * b + kp, cq * 512:(cq + 1) * 512],
                                st[0:cnt, :])

                eaw = stA.tile([128, 16, A], BF, name="eaw")
                for kt in range(16):
                    nc.sync.dma_start(eaw[:, kt, :], rr(encattwT_d)[:, kt, :])
                eab = stA.tile([128, A // 128], F32, name="eab")
                nc.sync.dma_start(eab[:], encattb_d[:])

                for half in range(2):
                    encT = stA.tile([128, S, 16, 128], BF, tag="encT")
                    for jj in range(S):
                        nc.sync.dma_start_transpose(encT[:, jj, :, :],
                                                    enc_sb[:, half * S + jj, :])
                    for at in range(4):
                        for ch in range(2):   # 4 j's each
                            pstile = psA.tile([128, 512], F32, tag="ps")
                            for et in range(16):
                                nc.tensor.matmul(
                                    pstile[:],
                                    lhsT=eaw[:, et, at * 128:(at + 1) * 128],
                                    rhs=encT[:, 4 * ch:4 * ch + 4, et, :],
                                    start=(et == 0), stop=(et == 15))
                            for jj in range(4):
                                j = half * S + 4 * ch + jj
                                b, kp = j // 2, j % 2
                                cnt = 128 if kp == 0 else P1
                                nc.vector.tensor_scalar(
                                    out=att1T[:, at, b * P + kp * 128:
                                              b * P + kp * 128 + cnt],
                                    in0=pstile[:, jj * 128:jj * 128 + cnt],
                                    scalar1=eab[:, at:at + 1], scalar2=None,
                                    op0=ALU.add)

            # ---------- Phase A2: mean, h0/c0, embedding, gates_emb ----------
            with ExitStack() as ph:
                stg = ph.enter_context(tc.tile_pool(name="stg2", bufs=2))
                st2 = ph.enter_context(tc.tile_pool(name="st2", bufs=1))
                psA = ph.enter_context(tc.tile_pool(name="psA2", bufs=4, space="PSUM"))
                psH = ph.enter_context(tc.tile_pool(name="psH", bufs=1, space="PSUM"))

                ones_bd = st2.tile([128, 8, 2, 4], BF, name="ones_bd")
                nc.vector.memset(ones_bd[:], 0.0)
                for bp in range(4):
                    for kp in range(2):
                        cnt = 128 if kp == 0 else P1
                        for g in range(2):
                            nc.vector.memset(
                                ones_bd[0:cnt, 2 * bp + kp, g, bp:bp + 1], 1.0 / P)
                mean_bfA = st2.tile([4, ENC], BF, name="mean_bfA")
                mean_bfB = st2.tile([4, ENC], BF, name="mean_bfB")
                for ch in range(4):
                    for g, mdst in ((0, mean_bfA), (1, mean_bfB)):
                        mp = psA.tile([4, 512], F32, tag="ps")
                        for kt in range(8):
                            nc.tensor.matmul(
                                mp[:],
                                lhsT=ones_bd[:, kt, g, :],
                                rhs=enc_sb[:, g * 8 + kt, ch * 512:(ch + 1) * 512],
                                start=(kt == 0), stop=(kt == 7))
                        nc.vector.tensor_copy(mdst[:, ch * 512:(ch + 1) * 512], mp[:])
                mean_T = st2.tile([128, 16, S], BF, name="mean_T")
                for half, msrc in ((0, mean_bfA), (1, mean_bfB)):
                    for et in range(16):
                        tp = psA.tile([128, 4], BF, tag="ps")
                        nc.tensor.transpose(tp[:], msrc[:, et * 128:(et + 1) * 128],
                                            ident[0:4, 0:4])
                        nc.vector.tensor_copy(mean_T[:, et, half * 4:half * 4 + 4],
                                              tp[:])

                ihc = st2.tile([128, 16, 2 * H], BF, name="ihc")
                for kt in range(16):
                    nc.sync.dma_start(ihc[:, kt, :], rr(inithcT_d)[:, kt, :])
                ib_bc = st2.tile([S, 2 * H], F32, name="ib_bc")
                nc.sync.dma_start(ib_bc[:], _bcast(initb_d[:], S))
                h0c0 = st2.tile([S, 2 * H], F32, name="h0c0")
                hcps = psH.tile([S, 2 * H], F32, name="hcps")
                for ch in range(2):
                    for kt in range(16):
                        nc.tensor.matmul(
                            hcps[:, ch * 512:(ch + 1) * 512],
                            lhsT=mean_T[:, kt, :],
                            rhs=ihc[:, kt, ch * 512:(ch + 1) * 512],
                            start=(kt == 0), stop=(kt == 15))
                nc.vector.tensor_tensor(out=h0c0[:], in0=hcps[:], in1=ib_bc[:],
                                        op=ALU.add)
                nc.vector.tensor_copy(c_st[:], h0c0[:, H:2 * H])
                h0bf = st2.tile([S, H], BF, name="h0bf")
                nc.vector.tensor_copy(h0bf[:], h0c0[:, 0:H])
                for kt in range(4):
                    tp = psA.tile([128, S], BF, tag="ps")
                    nc.tensor.transpose(tp[:], h0bf[:, kt * 128:(kt + 1) * 128],
                                        ident[0:S, 0:S])
                    nc.vector.tensor_copy(hT[:, kt, :], tp[:])

                # embedding gather -> embT -> gates_emb -> DRAM (bf16)
                wem = st2.tile([128, 4, 4 * H], BF, name="wem")
                for kt in range(4):
                    for hh in range(2):
                        nc.sync.dma_start(
                            wem[:, kt, hh * 1024:(hh + 1) * 1024],
                            rr(wembT_d)[:, kt, hh * 1024:(hh + 1) * 1024])
                bco = st2.tile([128, 4 * H], F32, name="bco")
                nc.sync.dma_start(bco[:], _bcast(bcomb_d[:], 128))
                embT = st2.tile([128, 4, SR], BF, name="embT")
                for mt, cnt in ((0, 128), (1, SR - 128)):
                    idx = stg.tile([128, 1], I32, tag="idx")
                    nc.sync.dma_start(idx[0:cnt, :], caps_d[mt * 128:mt * 128 + cnt, :])
                    eg = stg.tile([128, E], F32, tag="embg")
                    nc.gpsimd.indirect_dma_start(
                        out=eg[0:cnt, :], out_offset=None,
                        in_=embw_d[:],
                        in_offset=bass.IndirectOffsetOnAxis(ap=idx[0:cnt, 0:1], axis=0))
                    egb = stg.tile([128, E], BF, tag="embgb")
                    nc.vector.tensor_copy(egb[0:cnt, :], eg[0:cnt, :])
                    for et in range(4):
                        tp = psA.tile([128, 128], BF, tag="ps")
                        nc.tensor.transpose(tp[0:128, 0:cnt],
                                            egb[0:cnt, et * 128:(et + 1) * 128],
                                            ident[0:cnt, 0:cnt])
                        nc.vector.tensor_copy(embT[:, et, mt * 128:mt * 128 + cnt],
                                              tp[0:128, 0:cnt])
                for mt, cnt in ((0, 128), (1, SR - 128)):
                    for ch in range(4):
                        gp = psA.tile([128, 512], F32, tag="ps")
                        for kt in range(4):
                            nc.tensor.matmul(
                                gp[0:cnt, :],
                                lhsT=embT[:, kt, mt * 128:mt * 128 + cnt],
                                rhs=wem[:, kt, ch * 512:(ch + 1) * 512],
                                start=(kt == 0), stop=(kt == 3))
                        gesb = stg.tile([128, 512], BF, tag="gesb")
                        nc.vector.tensor_tensor(out=gesb[0:cnt, :], in0=gp[0:cnt, :],
                                                in1=bco[0:cnt, ch * 512:(ch + 1) * 512],
                                                op=ALU.add)
                        nc.sync.dma_start(
                            ge_dram[mt * 128:mt * 128 + cnt, ch * 512:(ch + 1) * 512],
                            gesb[0:cnt, :])

            # ---------- Phase W + recurrence ----------
            with ExitStack() as ph:
                wp = ph.enter_context(tc.tile_pool(name="wp", bufs=1))
                wenc = wp.tile([128, 16, 4 * H], BF, name="wenc")
                for kt in range(16):
                    for hh in range(2):
                        nc.sync.dma_start(
                            wenc[:, kt, hh * 1024:(hh + 1) * 1024],
                            rr(wencT_d)[:, kt, hh * 1024:(hh + 1) * 1024])
                whc = wp.tile([128, 4, 2 * ENC], BF, name="whc")
                for kt in range(4):
                    for hh in range(4):
                        nc.sync.dma_start(
                            whc[:, kt, hh * 1024:(hh + 1) * 1024],
                            rr(whcomb_d)[:, kt, hh * 1024:(hh + 1) * 1024])
                hat = wp.tile([128, 4, A], BF, name="hat")
                for kt in range(4):
                    nc.sync.dma_start(hat[:, kt, :], rr(hidattT_d)[:, kt, :])

                rsb = ph.enter_context(tc.tile_pool(name="rsb", bufs=1))
                sc5 = ph.enter_context(tc.tile_pool(name="sc5", bufs=4))
                scg = ph.enter_context(tc.tile_pool(name="scg", bufs=5))
                sc3 = ph.enter_context(tc.tile_pool(name="sc3", bufs=3))
                rb2 = ph.enter_context(tc.tile_pool(name="rb2", bufs=2))
                ps_awe = ph.enter_context(tc.tile_pool(name="ps_awe", bufs=2,
                                                       space="PSUM"))
                ps_tr = ph.enter_context(tc.tile_pool(name="ps_tr", bufs=2,
                                                      space="PSUM"))
                ps_g = ph.enter_context(tc.tile_pool(name="ps_g", bufs=2,
                                                     space="PSUM"))

                for t in range(T1):
                    mask8 = rsb.tile([S, 1], F32, name="mask8")
                    nc.sync.dma_start(mask8[:], masks_d[:, t:t + 1])
                    mask8i = rsb.tile([S, 1], I32, name="mask8i")
                    nc.sync.dma_start(mask8i[:], masksi_d[:, t:t + 1])

                    # att2T = hid_att_w @ h + hid_att_b   [A-part, b]
                    a2ps = ps_tr.tile([128, 4, S], F32, tag="tr")
                    for at in range(4):
                        for kt in range(4):
                            nc.tensor.matmul(
                                a2ps[:, at, :],
                                lhsT=hat[:, kt, at * 128:(at + 1) * 128],
                                rhs=hT[:, kt, :],
                                start=(kt == 0), stop=(kt == 3))
                    att2 = rsb.tile([128, 4, S], F32, name="att2")
                    for at in range(4):
                        nc.vector.tensor_scalar(
                            out=att2[:, at, :], in0=a2ps[:, at, :],
                            scalar1=hab[:, at:at + 1], scalar2=None, op0=ALU.add)

                    # gate pre-activation: fbeta part of whc (cols [0, 2048))
                    gpsl = []
                    for gh in range(2):
                        g1 = ps_g.tile([S, 1024], F32, tag="gps")
                        gpsl.append(g1)
                        for sub in range(2):
                            for kt in range(4):
                                nc.tensor.matmul(
                                    g1[:, sub * 512:(sub + 1) * 512],
                                    lhsT=hT[:, kt, :],
                                    rhs=whc[:, kt, gh * 1024 + sub * 512:
                                            gh * 1024 + (sub + 1) * 512],
                                    start=(kt == 0), stop=(kt == 3))

                    # relu(att1 + att2) and e-reduce (col-tiled m=1 per sample)
                    epsA = ps_awe.tile([128, 512], F32, tag="aweps")
                    epsB = ps_awe.tile([128, 512], F32, tag="aweps")
                    for b in range(S):
                        ep = epsA if b < 4 else epsB
                        j = b % 4
                        rb = rb2.tile([128, 4, P], BF, tag="rb")
                        for at in range(4):
                            nc.vector.tensor_scalar(
                                out=rb[:, at, :],
                                in0=att1T[:, at, b * P:(b + 1) * P],
                                scalar1=att2[:, at, b:b + 1], scalar2=0.0,
                                op0=ALU.add, op1=ALU.max)
                        for at in range(4):
                            nc.tensor.matmul(
                                ep[32 * j:32 * j + 1, 0:P],
                                lhsT=wfu[:, at, :], rhs=rb[:, at, :],
                                start=(at == 0), stop=(at == 3),
                                tile_position=(0, 32 * j))

                    # softmax: exp per psum row (32j base), DMA-assemble halves
                    exhalf = []
                    for half, ep in ((0, epsA), (1, epsB)):
                        ex = sc3.tile([4, P], F32, tag="soft")
                        exhalf.append(ex)
                        for j in range(4):
                            exr = rb2.tile([1, P], F32, tag="exr")
                            nc.scalar.activation(out=exr[:],
                                                 in_=ep[32 * j:32 * j + 1, 0:P],
                                                 func=AF.Exp, bias=float(full_att_b))
                            nc.sync.dma_start(ex[j:j + 1, :], exr[:])
                    for half, msk in ((0, maskA), (1, maskB)):
                        ex = exhalf[half]
                        zz = sc3.tile([4, 1], F32, tag="zr")
                        nc.vector.tensor_reduce(zz[:], ex[:], mybir.AxisListType.X,
                                                ALU.add)
                        rz = sc3.tile([4, 1], F32, tag="zr")
                        nc.vector.reciprocal(rz[:], zz[:])
                        ao = sc3.tile([4, P], F32, tag="soft")
                        nc.vector.tensor_scalar(
                            out=ao[:], in0=ex[:], scalar1=rz[:, 0:1],
                            scalar2=msk[:, t:t + 1], op0=ALU.mult, op1=ALU.mult)
                        nc.sync.dma_start(alph_d[half * 4:half * 4 + 4, t, :], ao[:])
                        ab = sc3.tile([4, P], BF, tag="soft")
                        nc.vector.tensor_scalar(
                            out=ab[:], in0=ex[:], scalar1=rz[:, 0:1],
                            scalar2=None, op0=ALU.mult)
                        t1p = ps_tr.tile([128, 4], BF, tag="tr")
                        nc.tensor.transpose(t1p[:], ab[:, 0:128], ident[0:4, 0:4])
                        nc.vector.tensor_copy(expcol[:, 0, half * 4:half * 4 + 4],
                                              t1p[:])
                        t2p = ps_tr.tile([128, 4], BF, tag="tr")
                        nc.tensor.transpose(t2p[0:P1, :], ab[:, 128:P],
                                            ident[0:4, 0:4])
                        nc.vector.tensor_copy(expcol[0:P1, 1, half * 4:half * 4 + 4],
                                              t2p[0:P1, :])
                    # scatter alpha columns into the block-diag lhsT
                    for g in range(2):
                        for bp in range(4):
                            nc.vector.tensor_copy(
                                bd4[:, 2 * bp:2 * bp + 2, g, bp],
                                expcol[:, :, 4 * g + bp])

                    # awe (col-tiled per sample) + gate sigma + x_enc chunks
                    for ch in range(4):
                        cs = slice(ch * 512, (ch + 1) * 512)
                        gps = gpsl[ch // 2]
                        gcol = slice((ch % 2) * 512, (ch % 2) * 512 + 512)
                        g8 = sc5.tile([S, 512], BF, tag="st5")
                        nc.scalar.activation(out=g8[:], in_=gps[:, gcol],
                                             func=AF.Sigmoid)
                        gB = sc5.tile([4, 512], BF, tag="st5")
                        nc.sync.dma_start(gB[:], g8[4:8, :])
                        apA = ps_awe.tile([4, 512], F32, tag="aweps")
                        apB = ps_awe.tile([4, 512], F32, tag="aweps")
                        for g, ap_ in ((0, apA), (1, apB)):
                            for kt in range(8):
                                nc.tensor.matmul(
                                    ap_[:],
                                    lhsT=bd4[:, kt, g, :],
                                    rhs=enc_sb[:, g * 8 + kt, cs],
                                    start=(kt == 0), stop=(kt == 7))
                        x8 = sc5.tile([S, 512], BF, tag="st5")
                        nc.vector.tensor_tensor(out=x8[0:4, :], in0=apA[:],
                                                in1=g8[0:4, :], op=ALU.mult)
                        xB = sc5.tile([4, 512], BF, tag="st5")
                        nc.vector.tensor_tensor(out=xB[:], in0=apB[:],
                                                in1=gB[:], op=ALU.mult)
                        nc.sync.dma_start(x8[4:8, :], xB[:])
                        for ei in range(4):
                            et = ch * 4 + ei
                            xtp = ps_tr.tile([128, S], BF, tag="tr")
                            nc.tensor.transpose(xtp[:], x8[:, ei * 128:(ei + 1) * 128],
                                                ident[0:S, 0:S])
                            nc.vector.tensor_copy(x_encT[:, et, :], xtp[:])

                    # gates = ge + h@whh.T + x_enc@wenc.T ; LSTM cell
                    gql = []
                    for gh in range(2):
                        gg = ps_g.tile([S, 1024], F32, tag="gps")
                        for sub in range(2):
                            cc = gh * 1024 + sub * 512
                            for kt in range(4):
                                nc.tensor.matmul(
                                    gg[:, sub * 512:(sub + 1) * 512],
                                    lhsT=hT[:, kt, :],
                                    rhs=whc[:, kt, 2048 + cc:2048 + cc + 512],
                                    start=(kt == 0), stop=False)
                            for kt in range(16):
                                nc.tensor.matmul(
                                    gg[:, sub * 512:(sub + 1) * 512],
                                    lhsT=x_encT[:, kt, :],
                                    rhs=wenc[:, kt, cc:cc + 512],
                                    start=False, stop=(kt == 15))
                        gql.append(gg)

                    def gq(i):  # i-th 512-wide quarter of gates, ge added
                        geq = rb2.tile([S, 512], BF, tag="geq")
                        nc.sync.dma_start(geq[:], ge_view[:, t, i * 512:(i + 1) * 512])
                        out = scg.tile([S, 512], F32, tag="gsc")
                        nc.vector.tensor_tensor(
                            out=out[:], in0=gql[i // 2][:, (i % 2) * 512:(i % 2) * 512 + 512],
                            in1=geq[:], op=ALU.add)
                        return out

                    si = scg.tile([S, H], F32, tag="gsc")
                    nc.scalar.activation(out=si[:], in_=gq(0)[:], func=AF.Sigmoid)
                    sf = scg.tile([S, H], F32, tag="gsc")
                    nc.scalar.activation(out=sf[:], in_=gq(1)[:], func=AF.Sigmoid)
                    t1_ = scg.tile([S, H], F32, tag="gsc")
                    nc.vector.tensor_tensor(out=t1_[:], in0=sf[:], in1=c_st[:],
                                            op=ALU.mult)
                    tg = scg.tile([S, H], F32, tag="gsc")
                    nc.scalar.activation(out=tg[:], in_=gq(2)[:], func=AF.Tanh)
                    t2_ = scg.tile([S, H], F32, tag="gsc")
                    nc.vector.tensor_tensor(out=t2_[:], in0=si[:], in1=tg[:],
                                            op=ALU.mult)
                    cn = scg.tile([S, H], F32, tag="gsc")
                    nc.vector.tensor_tensor(out=cn[:], in0=t1_[:], in1=t2_[:],
                                            op=ALU.add)
                    so = scg.tile([S, H], F32, tag="gsc")
                    nc.scalar.activation(out=so[:], in_=gq(3)[:], func=AF.Sigmoid)
                    tcn = scg.tile([S, H], F32, tag="gsc")
                    nc.scalar.activation(out=tcn[:], in_=cn[:], func=AF.Tanh)
                    hn = scg.tile([S, H], F32, tag="gsc")
                    nc.vector.tensor_tensor(out=hn[:], in0=so[:], in1=tcn[:],
                                            op=ALU.mult)
                    nc.vector.copy_predicated(c_st[:], mask8i[:].to_broadcast([S, H]),
                                              cn[:])
                    hp = scg.tile([S, H], BF, tag="gsc")
                    nc.vector.tensor_scalar(out=hp[:], in0=hn[:],
                                            scalar1=mask8[:, 0:1],
                                            scalar2=None, op0=ALU.mult)
                    for kt in range(4):
                        htp = ps_tr.tile([128, S], BF, tag="tr")
                        nc.tensor.transpose(htp[:], hp[:, kt * 128:(kt + 1) * 128],
                                            ident[0:S, 0:S])
                        nc.vector.tensor_copy(H_allT[:, kt, t::T1], htp[:])
                    for kt in range(4):
                        nc.vector.copy_predicated(hT[:, kt, :], maskTb[:, t, :],
                                                  H_allT[:, kt, t::T1])

        # ---------------- Phase F: fc projection ----------------
        with ExitStack() as ph:
            fcp = ph.enter_context(tc.tile_pool(name="fcp", bufs=3))
            fst = ph.enter_context(tc.tile_pool(name="fst", bufs=3))
            fps = ph.enter_context(tc.tile_pool(name="fps", bufs=4, space="PSUM"))
            fone = ph.enter_context(tc.tile_pool(name="fone", bufs=1))

            if nz_fcb:
                mrow = fone.tile([1, SR], BF, name="mrow")
                nc.sync.dma_start(mrow[:], maskrow_d[:])
                fcb_sb = fone.tile([1, V], BF, name="fcb_sb")
                nc.sync.dma_start(fcb_sb[:], fcb_d[:])

            GW = 2048
            for g0 in range(0, V, GW):
                gw = min(GW, V - g0)
                fw = fcp.tile([128, 4, GW], BF, tag="fw")
                for kt in range(4):
                    for h0_ in range(0, gw, 1024):
                        hw_ = min(1024, gw - h0_)
                        nc.sync.dma_start(
                            fw[:, kt, h0_:h0_ + hw_],
                            rr(fcT_d)[:, kt, g0 + h0_:g0 + h0_ + hw_])
                for s0 in range(0, gw, 512):
                    sw = min(512, gw - s0)
                    for mt, cnt in ((0, 128), (1, SR - 128)):
                        pp = fps.tile([128, 512], F32, tag="fpp")
                        for kt in range(4):
                            nc.tensor.matmul(
                                pp[0:cnt, 0:sw],
                                lhsT=H_allT[:, kt, mt * 128:mt * 128 + cnt],
                                rhs=fw[:, kt, s0:s0 + sw],
                                start=(kt == 0), stop=(kt == 3 and not nz_fcb))
                        if nz_fcb:
                            nc.tensor.matmul(
                                pp[0:cnt, 0:sw],
                                lhsT=mrow[:, mt * 128:mt * 128 + cnt],
                                rhs=fcb_sb[:, g0 + s0:g0 + s0 + sw],
                                start=False, stop=True)
                        ps_out = fst.tile([128, 512], F32, tag="ps_out")
                        nc.scalar.copy(out=ps_out[0:cnt, 0:sw], in_=pp[0:cnt, 0:sw])
                        nc.sync.dma_start(
                            preds_d[mt * 128:mt * 128 + cnt, g0 + s0:g0 + s0 + sw],
                            ps_out[0:cnt, 0:sw])
    nc.compile()
    return nc


def _prep_inputs(inputs):
    bfl = ml_dtypes.bfloat16
    f = {k: np.asarray(v) for k, v in inputs.items()}
    caps = f["captions"].astype(np.int32)
    dec_len = f["lengths"].astype(np.int32) - 1
    masks_full = (np.arange(T1)[None, :] < dec_len[:, None]).astype(np.float32)

    def col_tiled(v):
        return np.ascontiguousarray(v.reshape(-1, 128).T)

    wih = f["lstm_w_ih"]
    shared = {
        "embw": np.ascontiguousarray(f["embed_w"], np.float32),
        "encattwT": np.ascontiguousarray(f["enc_att_w"].T.astype(bfl)),
        "encattb": col_tiled(f["enc_att_b"].astype(np.float32)),
        "hidattT": np.ascontiguousarray(f["hid_att_w"].T.astype(bfl)),
        "hidattb": col_tiled(f["hid_att_b"].astype(np.float32)),
        "wfull": col_tiled(f["full_att_w"][0].astype(bfl)),
        "inithcT": np.ascontiguousarray(
            np.concatenate([f["init_h_w"], f["init_c_w"]], 0).T.astype(bfl)),
        "initb": np.concatenate([f["init_h_b"], f["init_c_b"]]).astype(np.float32),
        "wembT": np.ascontiguousarray(wih[:, :E].T.astype(bfl)),
        "bcomb": (f["lstm_b_ih"] + f["lstm_b_hh"]).astype(np.float32),
        "wencT": np.ascontiguousarray(wih[:, E:].T.astype(bfl)),
        "whcomb": np.ascontiguousarray(
            np.concatenate([f["fbeta_w"].T, f["lstm_w_hh"].T], 1).astype(bfl)),
        "fcT": np.ascontiguousarray(f["fc_w"].T.astype(bfl)),
        "fcb": np.ascontiguousarray(f["fc_b"][None, :].astype(bfl)),
    }
    in_maps = []
    for c in range(NC_N):
        sl = slice(c * S, (c + 1) * S)
        m = masks_full[sl]
        d = dict(shared)
        d["enc"] = np.ascontiguousarray(f["encoder_out"][sl], np.float32)
        d["caps"] = np.ascontiguousarray(caps[sl, :T1].reshape(SR, 1))
        d["masks"] = np.ascontiguousarray(m)
        d["masktr"] = np.ascontiguousarray(m.T.astype(bfl))
        d["masksi"] = np.ascontiguousarray(m.astype(np.int32))
        d["masktri"] = np.ascontiguousarray(m.T.astype(np.int8))
        d["maskrow"] = np.ascontiguousarray(m.reshape(1, SR).astype(bfl))
        in_maps.append(d)
    return in_maps, float(np.asarray(inputs["full_att_b"])[0]), bool(np.any(f["fc_b"]))


def kernel_traced(trace=False, **inputs):
    from concourse.bass_utils import run_bass_kernel_spmd
    in_maps, fab, nz_fcb = _prep_inputs(inputs)
    nc = build(fab, nz_fcb)
    res = run_bass_kernel_spmd(nc, in_maps, core_ids=list(range(NC_N)), trace=trace)
    preds = np.zeros((B, T1, V), np.float32)
    alphas = np.zeros((B, T1, P), np.float32)
    for c, r in enumerate(res.results):
        preds[c * S:(c + 1) * S] = np.asarray(r["preds"]).reshape(S, T1, V)
        alphas[c * S:(c + 1) * S] = np.asarray(r["alph"])
    return preds, alphas, res


def kernel(**inputs):
    preds, alphas, _ = kernel_traced(trace=False, **inputs)
    return preds, alphas


# revision 26
# speedup vs baseline: 1.0291x; 1.0291x over previous
"""Trainium2 Bass kernel for DecoderRNNWithAttention.

Data-parallel over batch (8 samples/core x 8 cores), weights replicated,
bf16 matmul inputs with f32 PSUM accumulation, elementwise in f32.
Per core: startup (enc load/cast, xbar transpose, att1 GEMM, mean/h0/c0,
embedding gather + emb-gate GEMM staged to DRAM), 19 recurrence steps,
then a batched fc projection over all (b,t) rows with streamed fc weights.
"""
import os
# The Bass SPMD runner needs the 8 axon NeuronCores visible to jax; a
# JAX_PLATFORMS=cpu pin (common for reference-side code) would hide them.
if os.environ.get("JAX_PLATFORMS") == "cpu":
    del os.environ["JAX_PLATFORMS"]

import numpy as np
import ml_dtypes
from contextlib import ExitStack

import concourse.bass as bass
import concourse.bacc as bacc
import concourse.tile as tile
import concourse.mybir as mybir
from concourse.masks import make_identity

BF = mybir.dt.bfloat16
F32 = mybir.dt.float32
I32 = mybir.dt.int32
I8 = mybir.dt.int8
AF = mybir.ActivationFunctionType
ALU = mybir.AluOpType

B, P, ENC, H, A, E, V, T = 64, 196, 2048, 512, 512, 512, 10000, 20
NC_N = 8
S = B // NC_N          # 8 samples per core
T1 = T - 1             # 19 steps
SR = S * T1            # 152 (b, t) rows, b-major t-fast
P1 = P - 128           # 68


def _bcast(d_ap, nparts):
    return bass.AP(tensor=d_ap.tensor, offset=d_ap.offset,
                   ap=[[0, nparts]] + [list(p) for p in d_ap.ap])


def build(full_att_b: float, nz_fcb: bool, reps: int = 1):
    nc = bacc.Bacc("TRN2", target_bir_lowering=False)

    enc_d = nc.dram_tensor("enc", [S, P, ENC], F32, kind="ExternalInput")
    caps_d = nc.dram_tensor("caps", [SR, 1], I32, kind="ExternalInput")
    masks_d = nc.dram_tensor("masks", [S, T1], F32, kind="ExternalInput")
    masktr_d = nc.dram_tensor("masktr", [T1, S], BF, kind="ExternalInput")
    masksi_d = nc.dram_tensor("masksi", [S, T1], I32, kind="ExternalInput")
    masktri_d = nc.dram_tensor("masktri", [T1, S], I8, kind="ExternalInput")
    maskrow_d = nc.dram_tensor("maskrow", [1, SR], BF, kind="ExternalInput")
    embw_d = nc.dram_tensor("embw", [V, E], F32, kind="ExternalInput")
    encattwT_d = nc.dram_tensor("encattwT", [ENC, A], BF, kind="ExternalInput")
    encattb_d = nc.dram_tensor("encattb", [128, A // 128], F32, kind="ExternalInput")
    hidattT_d = nc.dram_tensor("hidattT", [H, A], BF, kind="ExternalInput")
    hidattb_d = nc.dram_tensor("hidattb", [128, A // 128], F32, kind="ExternalInput")
    wfull_d = nc.dram_tensor("wfull", [128, A // 128], BF, kind="ExternalInput")
    inithcT_d = nc.dram_tensor("inithcT", [ENC, 2 * H], BF, kind="ExternalInput")
    initb_d = nc.dram_tensor("initb", [2 * H], F32, kind="ExternalInput")
    wembT_d = nc.dram_tensor("wembT", [E, 4 * H], BF, kind="ExternalInput")
    bcomb_d = nc.dram_tensor("bcomb", [4 * H], F32, kind="ExternalInput")
    wencT_d = nc.dram_tensor("wencT", [ENC, 4 * H], BF, kind="ExternalInput")
    whcomb_d = nc.dram_tensor("whcomb", [H, 2 * ENC], BF, kind="ExternalInput")
    fcT_d = nc.dram_tensor("fcT", [H, V], BF, kind="ExternalInput")
    fcb_d = nc.dram_tensor("fcb", [1, V], BF, kind="ExternalInput")

    preds_d = nc.dram_tensor("preds", [SR, V], F32, kind="ExternalOutput")
    alph_d = nc.dram_tensor("alph", [S, T1, P], F32, kind="ExternalOutput")

    rr = lambda d: d[:].rearrange("(kt q) n -> q kt n", q=128)

    with tile.TileContext(nc) as tc, ExitStack() as top:
        glob = top.enter_context(tc.tile_pool(name="glob", bufs=1))
        dramp = top.enter_context(tc.tile_pool(name="dramp", bufs=1, space="DRAM"))
        ident = glob.tile([128, 128], BF, name="ident")
        make_identity(nc, ident)
        H_allT = glob.tile([128, 4, SR], BF, name="H_allT")
        ge_dram = dramp.tile([SR, 4 * H], BF, name="ge_dram")
        ge_view = ge_dram[:].rearrange("(b t) n -> b t n", t=T1)

        with ExitStack() as rctx:  # pools that live startup..recurrence-end
            pe = rctx.enter_context(tc.tile_pool(name="pe", bufs=1))
            enc_sb = pe.tile([128, 2 * S, ENC], BF, name="enc_sb")
            att1T = pe.tile([128, 4, S * P], BF, name="att1T")
            hT = pe.tile([128, 4, S], BF, name="hT")
            c_st = pe.tile([S, H], F32, name="c_st")
            expcol = pe.tile([128, 2, S], BF, name="expcol")
            x_encT = pe.tile([128, 16, S], BF, name="x_encT")
            maskTb = pe.tile([128, T1, S], I8, name="maskTb")
            maskA = pe.tile([4, T1], F32, name="maskA")
            maskB = pe.tile([4, T1], F32, name="maskB")
            wfu = pe.tile([128, 4, 1], BF, name="wfu")
            hab = pe.tile([128, A // 128], F32, name="hab")
            # block-diag alpha lhsT per 4-sample group: [q, kt(8), g(2), b'(4)]
            bd4 = pe.tile([128, 8, 2, 4], BF, name="bd4")
            nc.vector.memset(bd4[:], 0.0)

            nc.sync.dma_start(maskTb[:], _bcast(masktri_d[:], 128))
            nc.sync.dma_start(maskA[:], masks_d[0:4, :])
            nc.sync.dma_start(maskB[:], masks_d[4:8, :])
            nc.sync.dma_start(wfu[:], wfull_d[:, :, None])
            nc.sync.dma_start(hab[:], hidattb_d[:])
            nc.vector.memset(enc_sb[:], 0.0)
            nc.vector.memset(expcol[:], 0.0)

            # ---------- Phase A1: enc load/cast, encT halves, att1 ----------
            with ExitStack() as ph:
                stg = ph.enter_context(tc.tile_pool(name="stg", bufs=2))
                stA = ph.enter_context(tc.tile_pool(name="stA", bufs=1))
                psA = ph.enter_context(tc.tile_pool(name="psA", bufs=4, space="PSUM"))

                for b in range(S):
                    for kp in range(2):
                        cnt = 128 if kp == 0 else P1
                        for cq in range(4):
                            st = stg.tile([128, 512], F32, tag="encstage")
                            nc.sync.dma_start(
                                st[0:cnt, :],
                                enc_d[b, kp * 128:kp * 128 + cnt,
                                      cq * 512:(cq + 1) * 512])
                            nc.vector.tensor_copy(
                                enc_sb[0:cnt, 2 * b + kp, cq * 512:(cq + 1) * 512],
                                st[0:cnt, :])

                eaw = stA.tile([128, 16, A], BF, name="eaw")
                for kt in range(16):
                    nc.sync.dma_start(eaw[:, kt, :], rr(encattwT_d)[:, kt, :])
                eab = stA.tile([128, A // 128], F32, name="eab")
                nc.sync.dma_start(eab[:], encattb_d[:])

                for half in range(2):
                    encT = stA.tile([128, S, 16, 128], BF, tag="encT")
                    for jj in range(S):
                        nc.sync.dma_start_transpose(encT[:, jj, :, :],
                                                    enc_sb[:, half * S + jj, :])
                    for at in range(4):
                        for ch in range(2):   # 4 j's each
                            pstile = psA.tile([128, 512], F32, tag="ps")
                            for et in range(16):
                                nc.tensor.matmul(
                                    pstile[:],
                                    lhsT=eaw[:, et, at * 128:(at + 1) * 128],
                                    rhs=encT[:, 4 * ch:4 * ch + 4, et, :],
                                    start=(et == 0), stop=(et == 15))
                            for jj in range(4):
                                j = half * S + 4 * ch + jj
                                b, kp = j // 2, j % 2
                                cnt = 128 if kp == 0 else P1
                                nc.vector.tensor_scalar(
                                    out=att1T[:, at, b * P + kp * 128:
                                              b * P + kp * 128 + cnt],
                                    in0=pstile[:, jj * 128:jj * 128 + cnt],
                                    scalar1=eab[:, at:at + 1], scalar2=None,
                                    op0=ALU.add)

            # ---------- Phase A2: mean, h0/c0, embedding, gates_emb ----------
            with ExitStack() as ph:
                stg = ph.enter_context(tc.tile_pool(name="stg2", bufs=2))
                st2 = ph.enter_context(tc.tile_pool(name="st2", bufs=1))
                psA = ph.enter_context(tc.tile_pool(name="psA2", bufs=4, space="PSUM"))
                psH = ph.enter_context(tc.tile_pool(name="psH", bufs=1, space="PSUM"))

                ones_bd = st2.tile([128, 8, 2, 4], BF, name="ones_bd")
                nc.vector.memset(ones_bd[:], 0.0)
                for bp in range(4):
                    for kp in range(2):
                        cnt = 128 if kp == 0 else P1
                        for g in range(2):
                            nc.vector.memset(
                                ones_bd[0:cnt, 2 * bp + kp, g, bp:bp + 1], 1.0 / P)
                mean_bfA = st2.tile([4, ENC], BF, name="mean_bfA")
                mean_bfB = st2.tile([4, ENC], BF, name="mean_bfB")
                for ch in range(4):
                    for g, mdst in ((0, mean_bfA), (1, mean_bfB)):
                        mp = psA.tile([4, 512], F32, tag="ps")
                        for kt in range(8):
                            nc.tensor.matmul(
                                mp[:],
                                lhsT=ones_bd[:, kt, g, :],
                                rhs=enc_sb[:, g * 8 + kt, ch * 512:(ch + 1) * 512],
                                start=(kt == 0), stop=(kt == 7))
                        nc.vector.tensor_copy(mdst[:, ch * 512:(ch + 1) * 512], mp[:])
                mean_T = st2.tile([128, 16, S], BF, name="mean_T")
                for half, msrc in ((0, mean_bfA), (1, mean_bfB)):
                    for et in range(16):
                        tp = psA.tile([128, 4], BF, tag="ps")
                        nc.tensor.transpose(tp[:], msrc[:, et * 128:(et + 1) * 128],
                                            ident[0:4, 0:4])
                        nc.vector.tensor_copy(mean_T[:, et, half * 4:half * 4 + 4],
                                              tp[:])

                ihc = st2.tile([128, 16, 2 * H], BF, name="ihc")
                for kt in range(16):
                    nc.sync.dma_start(ihc[:, kt, :], rr(inithcT_d)[:, kt, :])
                ib_bc = st2.tile([S, 2 * H], F32, name="ib_bc")
                nc.sync.dma_start(ib_bc[:], _bcast(initb_d[:], S))
                h0c0 = st2.tile([S, 2 * H], F32, name="h0c0")
                hcps = psH.tile([S, 2 * H], F32, name="hcps")
                for ch in range(2):
                    for kt in range(16):
                        nc.tensor.matmul(
                            hcps[:, ch * 512:(ch + 1) * 512],
                            lhsT=mean_T[:, kt, :],
                            rhs=ihc[:, kt, ch * 512:(ch + 1) * 512],
                            start=(kt == 0), stop=(kt == 15))
                nc.vector.tensor_tensor(out=h0c0[:], in0=hcps[:], in1=ib_bc[:],
                                        op=ALU.add)
                nc.vector.tensor_copy(c_st[:], h0c0[:, H:2 * H])
                h0bf = st2.tile([S, H], BF, name="h0bf")
                nc.vector.tensor_copy(h0bf[:], h0c0[:, 0:H])
                for kt in range(4):
                    tp = psA.tile([128, S], BF, tag="ps")
                    nc.tensor.transpose(tp[:], h0bf[:, kt * 128:(kt + 1) * 128],
                                        ident[0:S, 0:S])
                    nc.vector.tensor_copy(hT[:, kt, :], tp[:])

                # embedding gather -> embT -> gates_emb -> DRAM (bf16)
                wem = st2.tile([128, 4, 4 * H], BF, name="wem")
                for kt in range(4):
                    for hh in range(2):
                        nc.sync.dma_start(
                            wem[:, kt, hh * 1024:(hh + 1) * 1024],
                            rr(wembT_d)[:, kt, hh * 1024:(hh + 1) * 1024])
                bco = st2.tile([128, 4 * H], F32, name="bco")
                nc.sync.dma_start(bco[:], _bcast(bcomb_d[:], 128))
                embT = st2.tile([128, 4, SR], BF, name="embT")
                for mt, cnt in ((0, 128), (1, SR - 128)):
                    idx = stg.tile([128, 1], I32, tag="idx")
                    nc.sync.dma_start(idx[0:cnt, :], caps_d[mt * 128:mt * 128 + cnt, :])
                    eg = stg.tile([128, E], F32, tag="embg")
                    nc.gpsimd.indirect_dma_start(
                        out=eg[0:cnt, :], out_offset=None,
                        in_=embw_d[:],
                        in_offset=bass.IndirectOffsetOnAxis(ap=idx[0:cnt, 0:1], axis=0))
                    egb = stg.tile([128, E], BF, tag="embgb")
                    nc.vector.tensor_copy(egb[0:cnt, :], eg[0:cnt, :])
                    for et in range(4):
                        tp = psA.tile([128, 128], BF, tag="ps")
                        nc.tensor.transpose(tp[0:128, 0:cnt],
                                            egb[0:cnt, et * 128:(et + 1) * 128],
                                            ident[0:cnt, 0:cnt])
                        nc.vector.tensor_copy(embT[:, et, mt * 128:mt * 128 + cnt],
                                              tp[0:128, 0:cnt])
                for mt, cnt in ((0, 128), (1, SR - 128)):
                    for ch in range(4):
                        gp = psA.tile([128, 512], F32, tag="ps")
                        for kt in range(4):
                            nc.tensor.matmul(
                                gp[0:cnt, :],
                                lhsT=embT[:, kt, mt * 128:mt * 128 + cnt],
                                rhs=wem[:, kt, ch * 512:(ch + 1) * 512],
                                start=(kt == 0), stop=(kt == 3))
                        gesb = stg.tile([128, 512], BF, tag="gesb")
                        nc.vector.tensor_tensor(out=gesb[0:cnt, :], in0=gp[0:cnt, :],
                                                in1=bco[0:cnt, ch * 512:(ch + 1) * 512],
                                                op=ALU.add)
                        nc.sync.dma_start(
                            ge_dram[mt * 128:mt * 128 + cnt, ch * 512:(ch + 1) * 512],
                            gesb[0:cnt, :])

            # ---------- Phase W + recurrence ----------
            with ExitStack() as ph:
                wp = ph.enter_context(tc.tile_pool(name="wp", bufs=1))
                wenc = wp.tile([128, 16, 4 * H], BF, name="wenc")
                for kt in range(16):
                    for hh in range(2):
                        nc.sync.dma_start(
                            wenc[:, kt, hh * 1024:(hh + 1) * 1024],
                            rr(wencT_d)[:, kt, hh * 1024:(hh + 1) * 1024])
                whc = wp.tile([128, 4, 2 * ENC], BF, name="whc")
                for kt in range(4):
                    for hh in range(4):
                        nc.sync.dma_start(
                            whc[:, kt, hh * 1024:(hh + 1) * 1024],
                            rr(whcomb_d)[:, kt, hh * 1024:(hh + 1) * 1024])
                hat = wp.tile([128, 4, A], BF, name="hat")
                for kt in range(4):
                    nc.sync.dma_start(hat[:, kt, :], rr(hidattT_d)[:, kt, :])

                rsb = ph.enter_context(tc.tile_pool(name="rsb", bufs=1))
                sc5 = ph.enter_context(tc.tile_pool(name="sc5", bufs=7))
                scg = ph.enter_context(tc.tile_pool(name="scg", bufs=5))
                sc3 = ph.enter_context(tc.tile_pool(name="sc3", bufs=3))
                rb2 = ph.enter_context(tc.tile_pool(name="rb2", bufs=2))
                ps_awe = ph.enter_context(tc.tile_pool(name="ps_awe", bufs=2,
                                                       space="PSUM"))
                ps_tr = ph.enter_context(tc.tile_pool(name="ps_tr", bufs=2,
                                                      space="PSUM"))
                ps_g = ph.enter_context(tc.tile_pool(name="ps_g", bufs=2,
                                                     space="PSUM"))

                for t in [tt for _ in range(reps) for tt in range(T1)]:
                    mask8 = rsb.tile([S, 1], F32, name="mask8")
                    nc.sync.dma_start(mask8[:], masks_d[:, t:t + 1])
                    mask8i = rsb.tile([S, 1], I32, name="mask8i")
                    nc.sync.dma_start(mask8i[:], masksi_d[:, t:t + 1])

                    # att2T = hid_att_w @ h + hid_att_b   [A-part, b]
                    a2ps = ps_tr.tile([128, 4, S], F32, tag="tr")
                    for at in range(4):
                        for kt in range(4):
                            nc.tensor.matmul(
                                a2ps[:, at, :],
                                lhsT=hat[:, kt, at * 128:(at + 1) * 128],
                                rhs=hT[:, kt, :],
                                start=(kt == 0), stop=(kt == 3))
                    att2 = rsb.tile([128, 4, S], F32, name="att2")
                    for at in range(4):
                        nc.vector.tensor_scalar(
                            out=att2[:, at, :], in0=a2ps[:, at, :],
                            scalar1=hab[:, at:at + 1], scalar2=None, op0=ALU.add)

                    # gate pre-activation: fbeta part of whc (cols [0, 2048))
                    gpsl = []
                    for gh in range(2):
                        g1 = ps_g.tile([S, 1024], F32, tag="gps")
                        gpsl.append(g1)
                        for sub in range(2):
                            for kt in range(4):
                                nc.tensor.matmul(
                                    g1[:, sub * 512:(sub + 1) * 512],
                                    lhsT=hT[:, kt, :],
                                    rhs=whc[:, kt, gh * 1024 + sub * 512:
                                            gh * 1024 + (sub + 1) * 512],
                                    start=(kt == 0), stop=(kt == 3))

                    # hoist gate sigmas for gh0 chunks + whh matmuls (fill the
                    # PE bubble while the softmax/alpha chain runs)
                    g8s, gBs = {}, {}
                    for ch in (0, 1):
                        gcol = slice((ch % 2) * 512, (ch % 2) * 512 + 512)
                        g8 = sc5.tile([S, 512], BF, tag="st5")
                        nc.scalar.activation(out=g8[:], in_=gpsl[0][:, gcol],
                                             func=AF.Tanh, scale=0.5)
                        nc.vector.tensor_scalar(out=g8[:], in0=g8[:], scalar1=0.5,
                                                scalar2=0.5, op0=ALU.mult, op1=ALU.add)
                        gB = sc5.tile([4, 512], BF, tag="st5")
                        nc.sync.dma_start(gB[:], g8[4:8, :])
                        g8s[ch], gBs[ch] = g8, gB
                    gql = []
                    for gh in range(2):
                        gg = ps_g.tile([S, 1024], F32, tag="gps")
                        gql.append(gg)
                        for sub in range(2):
                            cc = gh * 1024 + sub * 512
                            for kt in range(4):
                                nc.tensor.matmul(
                                    gg[:, sub * 512:(sub + 1) * 512],
                                    lhsT=hT[:, kt, :],
                                    rhs=whc[:, kt, 2048 + cc:2048 + cc + 512],
                                    start=(kt == 0), stop=False)

                    # relu(att1 + att2) and e-reduce (col-tiled m=1 per sample)
                    epsA = ps_awe.tile([128, 512], F32, tag="aweps")
                    epsB = ps_awe.tile([128, 512], F32, tag="aweps")
                    for b in range(S):
                        ep = epsA if b < 4 else epsB
                        j = b % 4
                        rb = rb2.tile([128, 4, P], BF, tag="rb")
                        for at in range(4):
                            nc.vector.tensor_scalar(
                                out=rb[:, at, :],
                                in0=att1T[:, at, b * P:(b + 1) * P],
                                scalar1=att2[:, at, b:b + 1], scalar2=0.0,
                                op0=ALU.add, op1=ALU.max)
                        for at in range(4):
                            nc.tensor.matmul(
                                ep[32 * j:32 * j + 1, 0:P],
                                lhsT=wfu[:, at, :], rhs=rb[:, at, :],
                                start=(at == 0), stop=(at == 3),
                                tile_position=(0, 32 * j))

                    # softmax: exp per psum row (32j base), DMA-assemble halves
                    exhalf = []
                    for half, ep in ((0, epsA), (1, epsB)):
                        ex = sc3.tile([4, P], F32, tag="soft")
                        exhalf.append(ex)
                        for j in range(4):
                            exr = rb2.tile([1, P], F32, tag="exr")
                            nc.scalar.activation(out=exr[:],
                                                 in_=ep[32 * j:32 * j + 1, 0:P],
                                                 func=AF.Exp, bias=float(full_att_b))
                            nc.sync.dma_start(ex[j:j + 1, :], exr[:])
                    for half, msk in ((0, maskA), (1, maskB)):
                        ex = exhalf[half]
                        zz = sc3.tile([4, 1], F32, tag="zr")
                        nc.vector.tensor_reduce(zz[:], ex[:], mybir.AxisListType.X,
                                                ALU.add)
                        rz = sc3.tile([4, 1], F32, tag="zr")
                        nc.vector.reciprocal(rz[:], zz[:])
                        ao = sc3.tile([4, P], F32, tag="soft")
                        nc.vector.tensor_scalar(
                            out=ao[:], in0=ex[:], scalar1=rz[:, 0:1],
                            scalar2=msk[:, t:t + 1], op0=ALU.mult, op1=ALU.mult)
                        nc.sync.dma_start(alph_d[half * 4:half * 4 + 4, t, :], ao[:])
                        ab = sc3.tile([4, P], BF, tag="soft")
                        nc.vector.tensor_scalar(
                            out=ab[:], in0=ex[:], scalar1=rz[:, 0:1],
                            scalar2=None, op0=ALU.mult)
                        t1p = ps_tr.tile([128, 4], BF, tag="tr")
                        nc.tensor.transpose(t1p[:], ab[:, 0:128], ident[0:4, 0:4])
                        nc.vector.tensor_copy(expcol[:, 0, half * 4:half * 4 + 4],
                                              t1p[:])
                        t2p = ps_tr.tile([128, 4], BF, tag="tr")
                        nc.tensor.transpose(t2p[0:P1, :], ab[:, 128:P],
                                            ident[0:4, 0:4])
                        nc.vector.tensor_copy(expcol[0:P1, 1, half * 4:half * 4 + 4],
                                              t2p[0:P1, :])
                    # scatter alpha columns into the block-diag lhsT
                    for g in range(2):
                        for bp in range(4):
                            nc.vector.tensor_copy(
                                bd4[:, 2 * bp:2 * bp + 2, g, bp],
                                expcol[:, :, 4 * g + bp])

                    # awe (col-tiled per sample) + gate sigma + x_enc chunks
                    for ch in range(4):
                        cs = slice(ch * 512, (ch + 1) * 512)
                        gcol = slice((ch % 2) * 512, (ch % 2) * 512 + 512)
                        if ch in g8s:
                            g8, gB = g8s[ch], gBs[ch]
                        else:
                            g8 = sc5.tile([S, 512], BF, tag="st5")
                            nc.scalar.activation(out=g8[:], in_=gpsl[1][:, gcol],
                                                 func=AF.Tanh, scale=0.5)
                            nc.vector.tensor_scalar(out=g8[:], in0=g8[:], scalar1=0.5,
                                                    scalar2=0.5, op0=ALU.mult,
                                                    op1=ALU.add)
                            gB = sc5.tile([4, 512], BF, tag="st5")
                            nc.sync.dma_start(gB[:], g8[4:8, :])
                        apA = ps_awe.tile([4, 512], F32, tag="aweps")
                        apB = ps_awe.tile([4, 512], F32, tag="aweps")
                        for g, ap_ in ((0, apA), (1, apB)):
                            for kt in range(8):
                                nc.tensor.matmul(
                                    ap_[:],
                                    lhsT=bd4[:, kt, g, :],
                                    rhs=enc_sb[:, g * 8 + kt, cs],
                                    start=(kt == 0), stop=(kt == 7))
                        x8 = sc5.tile([S, 512], BF, tag="st5")
                        nc.vector.tensor_tensor(out=x8[0:4, :], in0=apA[:],
                                                in1=g8[0:4, :], op=ALU.mult)
                        xB = sc5.tile([4, 512], BF, tag="st5")
                        nc.vector.tensor_tensor(out=xB[:], in0=apB[:],
                                                in1=gB[:], op=ALU.mult)
                        nc.sync.dma_start(x8[4:8, :], xB[:])
                        for ei in range(4):
                            et = ch * 4 + ei
                            xtp = ps_tr.tile([128, S], BF, tag="tr")
                            nc.tensor.transpose(xtp[:], x8[:, ei * 128:(ei + 1) * 128],
                                                ident[0:S, 0:S])
                            nc.vector.tensor_copy(x_encT[:, et, :], xtp[:])

                    # gates += x_enc@wenc.T ; LSTM cell
                    for gh in range(2):
                        gg = gql[gh]
                        for sub in range(2):
                            cc = gh * 1024 + sub * 512
                            for kt in range(16):
                                nc.tensor.matmul(
                                    gg[:, sub * 512:(sub + 1) * 512],
                                    lhsT=x_encT[:, kt, :],
                                    rhs=wenc[:, kt, cc:cc + 512],
                                    start=False, stop=(kt == 15))

                    def gq(i):  # i-th 512-wide quarter of gates, ge added
                        geq = rb2.tile([S, 512], BF, tag="geq")
                        nc.sync.dma_start(geq[:], ge_view[:, t, i * 512:(i + 1) * 512])
                        out = scg.tile([S, 512], F32, tag="gsc")
                        nc.vector.tensor_tensor(
                            out=out[:], in0=gql[i // 2][:, (i % 2) * 512:(i % 2) * 512 + 512],
                            in1=geq[:], op=ALU.add)
                        return out

                    si = scg.tile([S, H], F32, tag="gsc")
                    nc.scalar.activation(out=si[:], in_=gq(0)[:], func=AF.Tanh, scale=0.5)
                    nc.vector.tensor_scalar(out=si[:], in0=si[:], scalar1=0.5, scalar2=0.5, op0=ALU.mult, op1=ALU.add)
                    sf = scg.tile([S, H], F32, tag="gsc")
                    nc.scalar.activation(out=sf[:], in_=gq(1)[:], func=AF.Tanh, scale=0.5)
                    nc.vector.tensor_scalar(out=sf[:], in0=sf[:], scalar1=0.5, scalar2=0.5, op0=ALU.mult, op1=ALU.add)
                    t1_ = scg.tile([S, H], F32, tag="gsc")
                    nc.vector.tensor_tensor(out=t1_[:], in0=sf[:], in1=c_st[:],
                                            op=ALU.mult)
                    tg = scg.tile([S, H], F32, tag="gsc")
                    nc.scalar.activation(out=tg[:], in_=gq(2)[:], func=AF.Tanh)
                    t2_ = scg.tile([S, H], F32, tag="gsc")
                    nc.vector.tensor_tensor(out=t2_[:], in0=si[:], in1=tg[:],
                                            op=ALU.mult)
                    cn = scg.tile([S, H], F32, tag="gsc")
                    nc.vector.tensor_tensor(out=cn[:], in0=t1_[:], in1=t2_[:],
                                            op=ALU.add)
                    so = scg.tile([S, H], F32, tag="gsc")
                    nc.scalar.activation(out=so[:], in_=gq(3)[:], func=AF.Tanh, scale=0.5)
                    nc.vector.tensor_scalar(out=so[:], in0=so[:], scalar1=0.5, scalar2=0.5, op0=ALU.mult, op1=ALU.add)
                    tcn = scg.tile([S, H], F32, tag="gsc")
                    nc.scalar.activation(out=tcn[:], in_=cn[:], func=AF.Tanh)
                    hn = scg.tile([S, H], F32, tag="gsc")
                    nc.vector.tensor_tensor(out=hn[:], in0=so[:], in1=tcn[:],
                                            op=ALU.mult)
                    nc.vector.copy_predicated(c_st[:], mask8i[:].to_broadcast([S, H]),
                                              cn[:])
                    hp = scg.tile([S, H], BF, tag="gsc")
                    nc.vector.tensor_scalar(out=hp[:], in0=hn[:],
                                            scalar1=mask8[:, 0:1],
                                            scalar2=None, op0=ALU.mult)
                    for kt in range(4):
                        htp = ps_tr.tile([128, S], BF, tag="tr")
                        nc.tensor.transpose(htp[:], hp[:, kt * 128:(kt + 1) * 128],
                                            ident[0:S, 0:S])
                        nc.vector.tensor_copy(H_allT[:, kt, t::T1], htp[:])
                    for kt in range(4):
                        nc.vector.copy_predicated(hT[:, kt, :], maskTb[:, t, :],
                                                  H_allT[:, kt, t::T1])

        # ---------------- Phase F: fc projection ----------------
        with ExitStack() as ph:
            fcp = ph.enter_context(tc.tile_pool(name="fcp", bufs=3))
            fst = ph.enter_context(tc.tile_pool(name="fst", bufs=3))
            fps = ph.enter_context(tc.tile_pool(name="fps", bufs=4, space="PSUM"))
            fone = ph.enter_context(tc.tile_pool(name="fone", bufs=1))

            if nz_fcb:
                mrow = fone.tile([1, SR], BF, name="mrow")
                nc.sync.dma_start(mrow[:], maskrow_d[:])
                fcb_sb = fone.tile([1, V], BF, name="fcb_sb")
                nc.sync.dma_start(fcb_sb[:], fcb_d[:])

            GW = 2048
            for g0 in range(0, V, GW):
                gw = min(GW, V - g0)
                fw = fcp.tile([128, 4, GW], BF, tag="fw")
                for kt in range(4):
                    for h0_ in range(0, gw, 1024):
                        hw_ = min(1024, gw - h0_)
                        nc.sync.dma_start(
                            fw[:, kt, h0_:h0_ + hw_],
                            rr(fcT_d)[:, kt, g0 + h0_:g0 + h0_ + hw_])
                for s0 in range(0, gw, 512):
                    sw = min(512, gw - s0)
                    for mt, cnt in ((0, 128), (1, SR - 128)):
                        pp = fps.tile([128, 512], F32, tag="fpp")
                        for kt in range(4):
                            nc.tensor.matmul(
                                pp[0:cnt, 0:sw],
                                lhsT=H_allT[:, kt, mt * 128:mt * 128 + cnt],
                                rhs=fw[:, kt, s0:s0 + sw],
                                start=(kt == 0), stop=(kt == 3 and not nz_fcb))
                        if nz_fcb:
                            nc.tensor.matmul(
                                pp[0:cnt, 0:sw],
                                lhsT=mrow[:, mt * 128:mt * 128 + cnt],
                                rhs=fcb_sb[:, g0 + s0:g0 + s0 + sw],
                                start=False, stop=True)
                        ps_out = fst.tile([128, 512], F32, tag="ps_out")
                        nc.scalar.copy(out=ps_out[0:cnt, 0:sw], in_=pp[0:cnt, 0:sw])
                        nc.sync.dma_start(
                            preds_d[mt * 128:mt * 128 + cnt, g0 + s0:g0 + s0 + sw],
                            ps_out[0:cnt, 0:sw])
    nc.compile()
    return nc


def _prep_inputs(inputs):
    bfl = ml_dtypes.bfloat16
    f = {k: np.asarray(v) for k, v in inputs.items()}
    caps = f["captions"].astype(np.int32)
    dec_len = f["lengths"].astype(np.int32) - 1
    masks_full = (np.arange(T1)[None, :] < dec_len[:, None]).astype(np.float32)

    def col_tiled(v):
        return np.ascontiguousarray(v.reshape(-1, 128).T)

    wih = f["lstm_w_ih"]
    shared = {
        "embw": np.ascontiguousarray(f["embed_w"], np.float32),
        "encattwT": np.ascontiguousarray(f["enc_att_w"].T.astype(bfl)),
        "encattb": col_tiled(f["enc_att_b"].astype(np.float32)),
        "hidattT": np.ascontiguousarray(f["hid_att_w"].T.astype(bfl)),
        "hidattb": col_tiled(f["hid_att_b"].astype(np.float32)),
        "wfull": col_tiled(f["full_att_w"][0].astype(bfl)),
        "inithcT": np.ascontiguousarray(
            np.concatenate([f["init_h_w"], f["init_c_w"]], 0).T.astype(bfl)),
        "initb": np.concatenate([f["init_h_b"], f["init_c_b"]]).astype(np.float32),
        "wembT": np.ascontiguousarray(wih[:, :E].T.astype(bfl)),
        "bcomb": (f["lstm_b_ih"] + f["lstm_b_hh"]).astype(np.float32),
        "wencT": np.ascontiguousarray(wih[:, E:].T.astype(bfl)),
        "whcomb": np.ascontiguousarray(
            np.concatenate([f["fbeta_w"].T, f["lstm_w_hh"].T], 1).astype(bfl)),
        "fcT": np.ascontiguousarray(f["fc_w"].T.astype(bfl)),
        "fcb": np.ascontiguousarray(f["fc_b"][None, :].astype(bfl)),
    }
    in_maps = []
    for c in range(NC_N):
        sl = slice(c * S, (c + 1) * S)
        m = masks_full[sl]
        d = dict(shared)
        d["enc"] = np.ascontiguousarray(f["encoder_out"][sl], np.float32)
        d["caps"] = np.ascontiguousarray(caps[sl, :T1].reshape(SR, 1))
        d["masks"] = np.ascontiguousarray(m)
        d["masktr"] = np.ascontiguousarray(m.T.astype(bfl))
        d["masksi"] = np.ascontiguousarray(m.astype(np.int32))
        d["masktri"] = np.ascontiguousarray(m.T.astype(np.int8))
        d["maskrow"] = np.ascontiguousarray(m.reshape(1, SR).astype(bfl))
        in_maps.append(d)
    return in_maps, float(np.asarray(inputs["full_att_b"])[0]), bool(np.any(f["fc_b"]))


def kernel_traced(trace=False, **inputs):
    from concourse.bass_utils import run_bass_kernel_spmd
    in_maps, fab, nz_fcb = _prep_inputs(inputs)
    nc = build(fab, nz_fcb)
    res = run_bass_kernel_spmd(nc, in_maps, core_ids=list(range(NC_N)), trace=trace)
    preds = np.zeros((B, T1, V), np.float32)
    alphas = np.zeros((B, T1, P), np.float32)
    for c, r in enumerate(res.results):
        preds[c * S:(c + 1) * S] = np.asarray(r["preds"]).reshape(S, T1, V)
        alphas[c * S:(c + 1) * S] = np.asarray(r["alph"])
    return preds, alphas, res


def kernel(**inputs):
    preds, alphas, _ = kernel_traced(trace=False, **inputs)
    return preds, alphas
